# revision 9
# baseline (speedup 1.0000x reference)
"""Fused multi-head attention for trn2, 8-core SPMD.

Problem: B=2, T=4096, C=768, H=12 heads, D=64. Sharding: 24 (batch, head)
pairs -> 3 heads per core (cores 0-3: batch 0, cores 4-7: batch 1). Each
core computes qkv projection for its heads, flash-style attention (no
max-subtraction: scores are ~N(0,1), exp is safe in fp32), and its partial
of the output projection; the host sums the 4 partials per batch.

Layout strategy (everything contracts on the partition dim):
  - host passes xT = x[b].T as [128, 6, 4096]  (c-chunk on partitions)
  - QT/KT computed as [d, t] "stacked pairs": group A rows 0:64 = head0,
    rows 64:128 = head1; group B = head2 duplicated. This feeds row-packed
    (tile_position auto) S^T matmuls with K=64 contraction at full array
    utilization.
  - S^T tiles [tk=128, tq=512] x 2 heads share one [128, 1024] PSUM tile,
    one exp() ACT op each (scale=0.125 fused, fp32 psum -> bf16 sbuf).
  - PV: out' = V^T-free col-packed matmuls, softmax denominators via
    M=1 ones-column matmuls into spare array columns.
  - normalize: PE-transpose O'^T -> [t, d], per-partition reciprocal scale,
    PE-transpose back -> c_proj lhsT; c_proj accumulates all 3 heads
    (+optional b_attn rank-1 terms) in PSUM, writes [t,768] fp32 partial.
"""

import numpy as np
import ml_dtypes
from contextlib import ExitStack

import concourse.bass as bass
import concourse.bacc as bacc
import concourse.mybir as mybir
import concourse.tile as tile
from concourse.masks import make_identity
from concourse.bass_utils import run_bass_kernel_spmd

F32 = mybir.dt.float32
BF16 = mybir.dt.bfloat16
BF = ml_dtypes.bfloat16

D_MODEL = 768
NHEAD = 12
HD = 64
B = 2
T = 4096
NCORES = 8
KC = D_MODEL // 128  # 6 contraction chunks for qkv proj
NTB = 4              # t-blocks of 1024 in phase 1
NQ = T // 512        # 8 tq-512 blocks
NK = T // 128        # 32 tk-128 chunks

_NC_CACHE = {}


def _build(has_battn: bool) -> bass.Bass:
    nc = bacc.Bacc()
    xT = nc.dram_tensor("xT", [128, KC, T], BF16, kind="ExternalInput")
    wqA = nc.dram_tensor("wqA", [128, KC, 128], BF16, kind="ExternalInput")
    wqB = nc.dram_tensor("wqB", [128, KC, 128], BF16, kind="ExternalInput")
    wkA = nc.dram_tensor("wkA", [128, KC, 128], BF16, kind="ExternalInput")
    wkB = nc.dram_tensor("wkB", [128, KC, 128], BF16, kind="ExternalInput")
    wv = nc.dram_tensor("wv", [128, KC, 192], BF16, kind="ExternalInput")
    wp2 = nc.dram_tensor("wp2", [128, 768], BF16, kind="ExternalInput")
    wp1 = nc.dram_tensor("wp1", [64, 768], BF16, kind="ExternalInput")
    if has_battn:
        bqA = nc.dram_tensor("bqA", [1, 128], BF16, kind="ExternalInput")
        bqB = nc.dram_tensor("bqB", [1, 128], BF16, kind="ExternalInput")
        bkA = nc.dram_tensor("bkA", [1, 128], BF16, kind="ExternalInput")
        bkB = nc.dram_tensor("bkB", [1, 128], BF16, kind="ExternalInput")
        bv = nc.dram_tensor("bv", [1, 192], BF16, kind="ExternalInput")
    y = nc.dram_tensor("y", [T, 768], F32, kind="ExternalOutput")

    with ExitStack() as ctx:
        tc = ctx.enter_context(tile.TileContext(nc))
        const = ctx.enter_context(tc.tile_pool(name="const", bufs=1))
        big = ctx.enter_context(tc.tile_pool(name="big", bufs=1))
        xs = ctx.enter_context(tc.tile_pool(name="xs", bufs=2))
        sb = ctx.enter_context(tc.tile_pool(name="sb", bufs=2))
        osbp = ctx.enter_context(tc.tile_pool(name="osbp", bufs=3))
        cst = ctx.enter_context(tc.tile_pool(name="cst", bufs=10))
        ysp = ctx.enter_context(tc.tile_pool(name="ysp", bufs=3))
        ps = ctx.enter_context(tc.tile_pool(name="ps", bufs=2, space="PSUM"))
        ps1 = ctx.enter_context(tc.tile_pool(name="ps1", bufs=1, space="PSUM"))

        # ---- constants ----
        wqA_s = const.tile([128, KC, 128], BF16, tag="wqA")
        wqB_s = const.tile([128, KC, 128], BF16, tag="wqB")
        wkA_s = const.tile([128, KC, 128], BF16, tag="wkA")
        wkB_s = const.tile([128, KC, 128], BF16, tag="wkB")
        wv_s = const.tile([128, KC, 192], BF16, tag="wv")
        wp2_s = const.tile([128, 768], BF16, tag="wp2")
        wp1_s = const.tile([64, 768], BF16, tag="wp1")
        for dst, src in [(wqA_s, wqA), (wqB_s, wqB), (wkA_s, wkA),
                         (wkB_s, wkB), (wv_s, wv), (wp2_s, wp2), (wp1_s, wp1)]:
            nc.sync.dma_start(out=dst, in_=src[:, :])
        bias_s = {}
        if has_battn:
            for name, src, w in [("bqA", bqA, 128), ("bqB", bqB, 128),
                                 ("bkA", bkA, 128), ("bkB", bkB, 128),
                                 ("bv", bv, 192)]:
                t = const.tile([1, w], BF16, tag=name)
                nc.sync.dma_start(out=t, in_=src[:, :])
                bias_s[name] = t
            ones_row = const.tile([1, 1024], BF16, tag="ones_row")
            nc.vector.memset(ones_row, 1.0)
        ones_col = const.tile([128, 1], BF16, tag="ones_col")
        nc.vector.memset(ones_col, 1.0)
        ident_f = const.tile([128, 128], F32, tag="ident_f")
        make_identity(nc, ident_f)
        ident_b = const.tile([128, 128], BF16, tag="ident_b")
        make_identity(nc, ident_b)

        # ---- persistent activations ----
        QTA = big.tile([128, T], BF16, tag="QTA")
        QTB = big.tile([128, T], BF16, tag="QTB")
        KTA = big.tile([128, T], BF16, tag="KTA")
        KTB = big.tile([128, T], BF16, tag="KTB")
        V = big.tile([128, NK, 192], BF16, tag="V")

        # ---- phase 1: qkv projection ----
        groups = [(QTA, wqA_s, "bqA"), (QTB, wqB_s, "bqB"),
                  (KTA, wkA_s, "bkA"), (KTB, wkB_s, "bkB")]
        for tb in range(NTB):
            xt = xs.tile([128, KC, 1024], BF16, tag="xt")
            nc.sync.dma_start(out=xt, in_=xT[:, :, tb * 1024:(tb + 1) * 1024])
            for dst, w_s, bname in groups:
                qp = ps.tile([128, 1024], F32, tag="s")
                for half in range(2):
                    o = qp[:, half * 512:(half + 1) * 512]
                    for j in range(KC):
                        nc.tensor.matmul(
                            o, w_s[:, j, :], xt[:, j, half * 512:(half + 1) * 512],
                            start=(j == 0), stop=(j == KC - 1 and not has_battn))
                    if has_battn:
                        nc.tensor.matmul(
                            o, bias_s[bname],
                            ones_row[:, half * 512:(half + 1) * 512],
                            start=False, stop=True)
                nc.vector.tensor_copy(
                    out=dst[:, tb * 1024:(tb + 1) * 1024], in_=qp)
            for tsub in range(8):
                t128 = tb * 8 + tsub
                vp = ps1.tile([128, 192], F32, tag="cpb")
                for j in range(KC):
                    nc.tensor.matmul(
                        vp, xt[:, j, tsub * 128:(tsub + 1) * 128], wv_s[:, j, :],
                        start=(j == 0), stop=(j == KC - 1 and not has_battn))
                if has_battn:
                    nc.tensor.matmul(vp, ones_row[:, 0:128], bias_s["bv"],
                                     start=False, stop=True)
                nc.vector.tensor_copy(out=V[:, t128, :], in_=vp)

        # ---- phase 2: attention + projection ----
        cA = {}
        cB = {}

        def attn_iter(KT, QT, qt0, qt1, va, vb):
            """Two row-packed heads: rg0 -> (va head cols, tq block qt0),
            rg1 -> (vb, qt1). Returns the two [65, 512] fp32 O'+sums sbuf
            tiles (rows 0:64 = O'^T, row 64 = softmax denominators)."""
            pv = ps1.tile([128, 512], F32, tag="pv")
            sm = ps1.tile([33, 512], F32, tag="sm")
            sps = []
            for k in range(NK + 1):
                if k < NK:
                    s = ps.tile([128, 1024], F32, tag="s")
                    nc.tensor.matmul(
                        s[:, 0:512], KT[0:64, k * 128:(k + 1) * 128],
                        QT[0:64, qt0 * 512:(qt0 + 1) * 512],
                        start=True, stop=True)
                    nc.tensor.matmul(
                        s[:, 512:1024], KT[64:128, k * 128:(k + 1) * 128],
                        QT[64:128, qt1 * 512:(qt1 + 1) * 512],
                        start=True, stop=True)
                    sps.append(s)
                if k == 0:
                    continue
                kk = k - 1
                s = sps[kk]
                pT = sb.tile([128, 1024], BF16, tag="pT")
                nc.scalar.activation(pT, s, mybir.ActivationFunctionType.Exp,
                                     scale=0.125)
                st, sp = (kk == 0), (kk == NK - 1)
                nc.tensor.matmul(pv[0:64, :], V[:, kk, va * 64:(va + 1) * 64],
                                 pT[:, 0:512], start=st, stop=sp,
                                 tile_position=(0, 0), skip_group_check=True)
                nc.tensor.matmul(pv[64:128, :], V[:, kk, vb * 64:(vb + 1) * 64],
                                 pT[:, 512:1024], start=st, stop=sp,
                                 tile_position=(0, 64), skip_group_check=True)
                nc.tensor.matmul(sm[0:1, :], ones_col, pT[:, 0:512],
                                 start=st, stop=sp, tile_position=(0, 0),
                                 skip_group_check=True)
                nc.tensor.matmul(sm[32:33, :], ones_col, pT[:, 512:1024],
                                 start=st, stop=sp, tile_position=(0, 32),
                                 skip_group_check=True)
            outs = []
            for h in range(2):
                osb = osbp.tile([65, 512], F32, tag="osb")
                nc.vector.tensor_copy(out=osb[0:64, :],
                                      in_=pv[h * 64:(h + 1) * 64, :])
                nc.vector.tensor_copy(out=osb[64:65, :],
                                      in_=sm[h * 32:h * 32 + 1, :])
                outs.append(osb)
            return outs

        def norm_chunks(osb, qh):
            """[65, 512] O'+sums -> 4 normalized [64, 128] bf16 lhsT psum
            tiles (one per t-128 chunk of the tq-512 block)."""
            res = []
            for t in range(4):
                tp = ps1.tile([128, 128], F32, tag="tp")
                nc.tensor.transpose(tp[:, 0:65], osb[:, t * 128:(t + 1) * 128],
                                    ident_f[0:65, 0:65])
                rs = sb.tile([128, 1], F32, tag="rs")
                nc.vector.reciprocal(rs, tp[:, 64:65])
                on = sb.tile([128, 64], BF16, tag="on")
                nc.vector.tensor_scalar_mul(on, tp[:, 0:64], rs)
                t2 = ps1.tile([64, 128], BF16, tag="tp")
                nc.tensor.transpose(t2, on, ident_b)
                res.append(t2)
            return res

        def cproj(qh):
            for t in range(4):
                t128 = qh * 4 + t
                cp = ps1.tile([128, 512], F32, tag="pv" if t % 2 == 0 else "sm")
                cpb = ps1.tile([128, 256], F32,
                               tag="cpb" if t % 2 == 0 else "tp")
                for o, n0, nw in ((cp, 0, 512), (cpb, 512, 256)):
                    nc.tensor.matmul(o, cA[(qh, t)], wp2_s[:, n0:n0 + nw],
                                     start=True, stop=False,
                                     skip_group_check=True)
                    nc.tensor.matmul(o, cB[(qh, t)], wp1_s[:, n0:n0 + nw],
                                     start=False, stop=True,
                                     skip_group_check=True)
                ysb = ysp.tile([128, 768], F32, tag="ysb")
                nc.vector.tensor_copy(out=ysb[:, 0:512], in_=cp)
                nc.vector.tensor_copy(out=ysb[:, 512:768], in_=cpb)
                nc.sync.dma_start(out=y[t128 * 128:(t128 + 1) * 128, :],
                                  in_=ysb)
                del cA[(qh, t)], cB[(qh, t)]

        for i in range(4):
            for qh in (2 * i, 2 * i + 1):
                o0, o1 = attn_iter(KTA, QTA, qh, qh, 0, 1)
                for h, osb in ((0, o0), (1, o1)):
                    for t, t2 in enumerate(norm_chunks(osb, qh)):
                        if h == 0:
                            c = cst.tile([128, 128], BF16, tag="cA")
                            cA[(qh, t)] = c
                        else:
                            c = cA[(qh, t)]
                        nc.vector.tensor_copy(out=c[h * 64:(h + 1) * 64, :],
                                              in_=t2)
            o0, o1 = attn_iter(KTB, QTB, 2 * i, 2 * i + 1, 2, 2)
            for qh, osb in ((2 * i, o0), (2 * i + 1, o1)):
                for t, t2 in enumerate(norm_chunks(osb, qh)):
                    c = cst.tile([64, 128], BF16, tag="cB")
                    nc.vector.tensor_copy(out=c, in_=t2)
                    cB[(qh, t)] = c
            cproj(2 * i)
            cproj(2 * i + 1)

    nc.compile()
    return nc


def _prep_inputs(x, W_attn, b_attn, W_proj, b_proj):
    """Shard to 8 per-core input dicts (host-side layout massaging)."""
    has_battn = bool(np.any(b_attn))

    def chunk6(w):  # [768, m] -> [128, 6, m]
        m = w.shape[1]
        return np.ascontiguousarray(
            w.reshape(KC, 128, m).transpose(1, 0, 2)).astype(BF)

    in_maps = []
    for c in range(NCORES):
        b = c // 4
        h0 = 3 * (c % 4)
        q = [W_attn[:, (h0 + i) * HD:(h0 + i + 1) * HD] for i in range(3)]
        k = [W_attn[:, 768 + (h0 + i) * HD:768 + (h0 + i + 1) * HD]
             for i in range(3)]
        v = [W_attn[:, 1536 + (h0 + i) * HD:1536 + (h0 + i + 1) * HD]
             for i in range(3)]
        xTr = np.ascontiguousarray(x[b].T)  # [768, 4096]
        m = {
            "xT": chunk6(xTr),
            "wqA": chunk6(np.concatenate([q[0], q[1]], axis=1)),
            "wqB": chunk6(np.concatenate([q[2], q[2]], axis=1)),
            "wkA": chunk6(np.concatenate([k[0], k[1]], axis=1)),
            "wkB": chunk6(np.concatenate([k[2], k[2]], axis=1)),
            "wv": chunk6(np.concatenate(v, axis=1)),
            "wp2": np.ascontiguousarray(
                W_proj[h0 * HD:(h0 + 2) * HD, :]).astype(BF),
            "wp1": np.ascontiguousarray(
                W_proj[(h0 + 2) * HD:(h0 + 3) * HD, :]).astype(BF),
        }
        if has_battn:
            bq = [b_attn[(h0 + i) * HD:(h0 + i + 1) * HD] for i in range(3)]
            bk = [b_attn[768 + (h0 + i) * HD:768 + (h0 + i + 1) * HD]
                  for i in range(3)]
            bv_ = [b_attn[1536 + (h0 + i) * HD:1536 + (h0 + i + 1) * HD]
                   for i in range(3)]
            m["bqA"] = np.concatenate([bq[0], bq[1]])[None, :].astype(BF)
            m["bqB"] = np.concatenate([bq[2], bq[2]])[None, :].astype(BF)
            m["bkA"] = np.concatenate([bk[0], bk[1]])[None, :].astype(BF)
            m["bkB"] = np.concatenate([bk[2], bk[2]])[None, :].astype(BF)
            m["bv"] = np.concatenate(bv_)[None, :].astype(BF)
        in_maps.append(m)
    return in_maps, has_battn


def get_nc(has_battn):
    if has_battn not in _NC_CACHE:
        _NC_CACHE[has_battn] = _build(has_battn)
    return _NC_CACHE[has_battn]


def kernel(x, W_attn, b_attn, W_proj, b_proj):
    x = np.asarray(x, np.float32)
    W_attn = np.asarray(W_attn, np.float32)
    b_attn = np.asarray(b_attn, np.float32)
    W_proj = np.asarray(W_proj, np.float32)
    b_proj = np.asarray(b_proj, np.float32)
    in_maps, has_battn = _prep_inputs(x, W_attn, b_attn, W_proj, b_proj)
    nc = get_nc(has_battn)
    res = run_bass_kernel_spmd(nc, in_maps, list(range(NCORES)))
    out = np.zeros((B, T, D_MODEL), np.float32)
    for c in range(NCORES):
        out[c // 4] += res.results[c]["y"]
    out += b_proj[None, None, :].astype(np.float32)
    return out
